# revision 24
# baseline (speedup 1.0000x reference)
"""Bidirectional 2-layer GRU + FC kernel for Trainium2 (8 NeuronCores).

Strategy:
  - Cores 0-3 run layer-0 FORWARD for batch slice [8p, 8p+8); cores 4-7 run
    layer-0 BACKWARD for the same slices (fed time-reversed x + backward
    weights via per-core in_maps; the device program is SPMD-uniform).
  - gx (input projections + biases) are precomputed with big fp32r matmuls
    into a row-major DRAM layout [128, T, MCH*BA] so each scan step loads one
    contiguous 384B/partition row.
  - The recurrence runs in transposed layout: gate rows on partitions,
    batch on the free dim.  Per step: 48 bf16 matmuls (w_hh.T chunks) +
    4 bias matmuls, ordered r/z-gates first so the sigmoid overlaps the
    n-gate matmuls.  Gate math is full-width (6 DVE ops + 3 ACTs) using
    h' = z*h + (1-z)*n with (1-z) = sigmoid(-x).  rz and n PSUM
    accumulators live in different banks so DVE reads don't serialize
    against PE writes.
  - A pairwise AllGather {i, i+4} shares the layer-0 trajectories; both pair
    members then redundantly compute layer-1 forward for their 8 batches
    (recurrence wall-time is batch-independent, so redundancy is free).
  - Layer-1 backward contributes only its t=T-1 state to the output, which
    takes a single step from h0=0.  FC bias is fused into an ACT Identity.
"""

import contextlib

import numpy as np

B, T_FULL, I_IN, H, C = 32, 512, 256, 512, 10
NCORES = 8
BA = 8          # batch per core
MCH = 12        # 3H / 128 gate-row chunks
KH = 4          # H / 128 contraction chunks

_PROGRAM_CACHE = {}


def _build(T):
    import concourse.bacc as bacc
    import concourse.mybir as mybir
    import concourse.tile as tile

    f32 = mybir.dt.float32
    f32r = mybir.dt.float32r
    bf16 = mybir.dt.bfloat16
    SIG = mybir.ActivationFunctionType.Sigmoid
    TANH = mybir.ActivationFunctionType.Tanh
    IDENT = mybir.ActivationFunctionType.Identity
    ALU = mybir.AluOpType

    TB = min(64, T)          # timestep block for the big matmul phases
    NTB = T // TB
    GW = MCH * BA            # 96 gx row width

    nc = bacc.Bacc("TRN2", target_bir_lowering=False, debug=False,
                   num_devices=NCORES)

    def inp(name, shape, dt=f32r):
        return nc.dram_tensor(name, shape, dt, kind="ExternalInput").ap()

    xT = inp("xT", [I_IN, T, BA])               # own batch slice, own time order
    wihT0 = inp("wihT0", [I_IN, 3 * H])         # own direction w_ih.T
    bias0 = inp("bias0", [1, 3 * H])            # b_ih + b_hh (rz); n part = b_ih_n
    bhn0 = inp("bhn0", [1, H], bf16)                  # b_hh n part
    whhT0 = inp("whhT0", [H, 3 * H], bf16)
    wih1T_f = inp("wih1T_f", [H, 3 * H], bf16)        # w_ih_l1f.T rows 0:H   (f0 input)
    wih1T_b = inp("wih1T_b", [H, 3 * H], bf16)        # w_ih_l1f.T rows H:2H  (b0 input)
    bias1 = inp("bias1", [1, 3 * H])
    bhn1 = inp("bhn1", [1, H], bf16)
    whh1T = inp("whh1T", [H, 3 * H], bf16)
    wih1bT = inp("wih1bT", [2 * H, 3 * H], bf16)      # w_ih_l1b.T
    bias1b_sc = inp("bias1b_sc", [128, MCH], f32)   # per m-chunk column
    bhn1b_sc = inp("bhn1b_sc", [128, KH], f32)
    fcwT = inp("fcwT", [2 * H, C])
    fcb = inp("fcb", [C, 1], f32)

    outT = nc.dram_tensor("outT", [C, BA], f32, kind="ExternalOutput").ap()

    with tile.TileContext(nc) as tc, contextlib.ExitStack() as ctx:
        # ---------------- DRAM scratch (Tile-tracked) ----------------
        dramp = ctx.enter_context(tc.tile_pool(name="dramp", bufs=1, space="DRAM"))
        gx0 = dramp.tile([128, T, GW], f32, tag="gx0")
        hbuf = dramp.tile([T, 128, KH * BA], bf16, tag="hbuf")
        # AllGather lands in per-chunk tiles (contiguous in/out required).
        # 7 x 64 rows + 48 + 16: the small final chunks shrink the exposed
        # gap between scan0's last step and layer-1 work that needs t=T-1.
        _ag_bounds = [(64 * i, 64) for i in range(7)] + [(448, 48), (496, 16)]
        ag_chunks = [
            (s, L, dramp.tile([2, L, 128, KH * BA], bf16, tag=f"agc{i}",
                              name=f"agc{i}"))
            for i, (s, L) in enumerate(_ag_bounds)]
        gx1f = dramp.tile([128, T, GW], f32, tag="gx1f")
        gx1b = dramp.tile([128, T, GW], f32, tag="gx1b")

        # ---------------- persistent SBUF (one pool, distinct tags) --------
        constp = ctx.enter_context(tc.tile_pool(name="constp", bufs=1))

        def const_tile(shape, dt, tag):
            return constp.tile(shape, dt, tag=tag, name=tag)

        whhT0_sb = const_tile([128, KH, 3 * H], bf16, "whhT0_sb")
        nc.sync.dma_start(whhT0_sb[:], whhT0.rearrange("(k p) m -> p k m", p=128))
        bias0_sb = const_tile([128, 3 * H], f32r, "bias0_sb")[0:1, :]
        nc.sync.dma_start(bias0_sb, bias0[:])
        # b_hh_n as a zero-padded [128, H] weight tile (data in partition 0
        # only): the bias matmul's LDWEIGHTS is then a full 128-col load
        # that pipelines like the others (a [1,128] load can't FWL).
        bhn0_sb = const_tile([128, H], bf16, "bhn0_sb")
        nc.vector.memset(bhn0_sb[:], 0.0)
        nc.sync.dma_start(bhn0_sb[0:1, :], bhn0[:])
        ones_big = const_tile([128, TB * BA], f32, "ones_big")[0:1, :]
        nc.vector.memset(ones_big, 1.0)
        ones_bf = const_tile([128, BA], bf16, "ones_bf")
        nc.vector.memset(ones_bf[:], 1.0)

        whh1_sb = const_tile([128, KH, 3 * H], bf16, "whh1_sb")
        nc.sync.dma_start(whh1_sb[:], whh1T.rearrange("(k p) m -> p k m", p=128))
        bhn1_sb = const_tile([128, H], bf16, "bhn1_sb")
        nc.vector.memset(bhn1_sb[:], 0.0)
        nc.sync.dma_start(bhn1_sb[0:1, :], bhn1[:])
        bias1_sb = const_tile([128, 3 * H], f32r, "bias1_sb")[0:1, :]
        nc.sync.dma_start(bias1_sb, bias1[:])

        b1b_sb = const_tile([128, MCH], f32, "b1b_sb")
        nc.sync.dma_start(b1b_sb[:], bias1b_sc[:])
        bhn1b_sb = const_tile([128, KH], f32, "bhn1b_sb")
        nc.sync.dma_start(bhn1b_sb[:], bhn1b_sc[:])
        fcw_sb = const_tile([128, 2 * KH, C], f32r, "fcw_sb")
        nc.sync.dma_start(fcw_sb[:], fcwT.rearrange("(k p) c -> p k c", p=128))
        fcb_sb = const_tile([128, 1], f32, "fcb_sb")[0:C, :]
        nc.sync.dma_start(fcb_sb, fcb[:])

        # h-state tiles for both scans + l1b results (long-lived)
        h0a = const_tile([128, KH * BA], bf16, "h0a")
        h0b = const_tile([128, KH * BA], bf16, "h0b")
        h1a = const_tile([128, KH * BA], bf16, "h1a")
        h1b_ = const_tile([128, KH * BA], bf16, "h1b_")
        gxl = const_tile([128, MCH * BA], f32, "gxl")
        rl = const_tile([128, 4 * BA], f32, "rl")
        zpl = const_tile([128, 4 * BA], f32, "zpl")
        n1l = const_tile([128, 4 * BA], f32, "n1l")
        ntl = const_tile([128, 4 * BA], f32, "ntl")
        h1bk = const_tile([128, 4 * BA], f32r, "h1bk")
        x1last = const_tile([128, 2 * KH, BA], bf16, "x1last")
        out_sb = const_tile([128, BA], f32, "out_sb")[0:C, :]

        # Scan-hot pools allocated LOW in SBUF (before the big phase
        # tiles) so the per-step DVE/ACT operands stay clear of the PE
        # weight-stream regions.
        scan_pools = {}
        for nm in ("s0", "s1"):
            scan_pools[nm] = (
                ctx.enter_context(tc.tile_pool(name=f"{nm}_gx", bufs=6)),
                ctx.enter_context(tc.tile_pool(name=f"{nm}_g", bufs=3)),
            )

        # ======== Phase machinery: gx0 / AllGather / gx1 as generators ======
        # All big-matmul phase work is chopped into quanta (one mm / copy /
        # DMA per yield) and pumped from inside the scan0 loop, filling the
        # PE idle window of each recurrence step and hiding the AllGather.
        wih0_sb = const_tile([128, 2, 3 * H], f32r, "wih0_sb")
        nc.sync.dma_start(wih0_sb[:],
                          wihT0.rearrange("(k p) m -> p k m", p=128))
        w1f_sb = const_tile([128, KH, 3 * H], bf16, "w1f_sb")
        nc.sync.dma_start(w1f_sb[:],
                          wih1T_f.rearrange("(k p) m -> p k m", p=128))
        w1b_sb = const_tile([128, KH, 3 * H], bf16, "w1b_sb")
        nc.sync.dma_start(w1b_sb[:],
                          wih1T_b.rearrange("(k p) m -> p k m", p=128))

        ph_stack = contextlib.ExitStack()
        ph_mv0 = ph_stack.enter_context(tc.tile_pool(name="ph_mv0", bufs=2))
        ph_mv1 = ph_stack.enter_context(tc.tile_pool(name="ph_mv1", bufs=3))
        ph_ps = ph_stack.enter_context(tc.tile_pool(name="ph_ps", bufs=2,
                                                    space="PSUM"))
        ph_ot = ph_stack.enter_context(tc.tile_pool(name="ph_ot", bufs=3))

        xT_r = xT.rearrange("(k p) t b -> p k t b", p=128)

        def _copy_m(ot, ps, m):
            # deferred by one m-group when emitted, so the producing
            # matmuls' sems have posted and this never head-of-line
            # blocks the DVE/ACT queue it lands on.
            dst = ot[:, :, BA * m:BA * (m + 1)]
            src = ps[:].rearrange("p (t b) -> p t b", t=TB)
            if m % 2 == 0:
                nc.vector.tensor_copy(dst, src)
            else:
                nc.scalar.copy(dst, src)

        def phase0_block(c):
            mv = ph_mv0.tile([128, 2, TB, BA], f32r, tag="mv")
            nc.sync.dma_start(mv[:], xT_r[:, :, c * TB:(c + 1) * TB, :])
            yield
            ot = ph_ot.tile([128, TB, GW], f32, tag="ot")
            prev = None
            for m in range(MCH):
                ps = ph_ps.tile([128, TB * BA], f32, tag="ps")
                for k in range(2):
                    nc.tensor.matmul(ps[:],
                                     wih0_sb[:, k, 128 * m:128 * (m + 1)],
                                     mv[:, k, :, :],
                                     start=(k == 0), stop=False)
                    yield
                nc.tensor.matmul(ps[:], bias0_sb[:, 128 * m:128 * (m + 1)],
                                 ones_big.bitcast(f32r),
                                 start=False, stop=True)
                yield
                if prev is not None:
                    _copy_m(ot, prev[0], prev[1])
                    yield
                prev = (ps, m)
            _copy_m(ot, prev[0], prev[1])
            yield
            nc.scalar.dma_start(gx0[:, c * TB:(c + 1) * TB, :], ot[:])
            yield

        def gx1_pass(pi, c):
            w_sb = w1f_sb if pi == 0 else w1b_sb
            dstbuf = gx1f if pi == 0 else gx1b
            with_bias = pi == 0
            mv = ph_mv1.tile([128, TB, KH * BA], bf16, tag="g1mv")
            # gather the 64-row block from the (possibly split) AG chunks
            r0 = c * TB
            for (s, L, agt) in ag_chunks:
                lo, hi = max(r0, s), min(r0 + TB, s + L)
                if lo < hi:
                    nc.sync.dma_start(
                        mv[:, lo - r0:hi - r0, :],
                        agt[pi, lo - s:hi - s].rearrange("t p b -> p t b"))
            yield
            ot = ph_ot.tile([128, TB, GW], f32, tag="ot")
            prev = None
            for m in range(MCH):
                ps = ph_ps.tile([128, TB * BA], f32, tag="ps")
                for k in range(KH):
                    nc.tensor.matmul(
                        ps[:], w_sb[:, k, 128 * m:128 * (m + 1)],
                        mv[:, :, BA * k:BA * (k + 1)],
                        start=(k == 0),
                        stop=(not with_bias and k == KH - 1))
                    yield
                if with_bias:
                    nc.tensor.matmul(
                        ps[:], bias1_sb[:, 128 * m:128 * (m + 1)],
                        ones_big.bitcast(f32r), start=False, stop=True)
                    yield
                if prev is not None:
                    _copy_m(ot, prev[0], prev[1])
                    yield
                prev = (ps, m)
            _copy_m(ot, prev[0], prev[1])
            yield
            nc.scalar.dma_start(
                dstbuf[:, c * TB:(c + 1) * TB, :], ot[:])
            yield

        def allgather_chunk(idx):
            s, L, agt = ag_chunks[idx]
            nc.gpsimd.collective_compute(
                "AllGather", ALU.bypass,
                replica_groups=[[0, 4], [1, 5], [2, 6], [3, 7]],
                ins=[hbuf[s:s + L]],
                outs=[agt[:]])

        def drain(gen):
            for _ in gen:
                pass

        # Static schedule: step index -> work pumped after that step's body.
        pump = {}

        def register(gen, t0, qps, total):
            steps = (total + qps - 1) // qps
            for i in range(steps):
                pump.setdefault(t0 + i, []).append((gen, qps))

        # phase0 blocks 0,1 run up-front (block 0 gates scan0 step 0).
        drain(phase0_block(0))
        drain(phase0_block(1))
        live_gens = []

        def track(gen):
            live_gens.append(gen)
            return gen

        for c in range(2, NTB):
            register(track(phase0_block(c)), 64 * (c - 2) + 24, 2, 52)
        ag_at = {s + L - 1: i for i, (s, L, _) in enumerate(ag_chunks)}
        for c in range(NTB - 1):
            t0 = 64 * c + 84 if c < 6 else 460
            register(track(gx1_pass(0, c)), t0, 2, 80)
            register(track(gx1_pass(1, c)), t0 + 20, 2, 70)

        def s0_interleave(t):
            for gen, n in pump.get(t, ()):
                for _ in range(n):
                    if next(gen, StopIteration) is StopIteration:
                        break
            if t in ag_at:
                allgather_chunk(ag_at[t])

        # ================ generic GRU scan (full-width pipeline) ============
        # PE order: r/z gate chunks (m 0..7) first, then n chunks (m 8..11)
        # + b_hh_n bias.  Full-width gate math:
        #   rz = sig(ps_rz + gx_rz); zp = sig(-(z_pre));
        #   n = tanh(ps_n * r + gx_n); h' = z*h + zp*n.
        def scan(h_even, h_odd, gx_rows, gx_rows2, whh_sb, bhn_sb,
                 store_h, name, interleave=None, warm=0):
            h_t = [h_even, h_odd]
            nc.vector.memset(h_t[0][:], 0.0)
            gxp, gp = scan_pools[name]
            with contextlib.ExitStack() as ss:
                psr_p = ss.enter_context(tc.tile_pool(
                    name=f"{name}_psr", bufs=2, space="PSUM"))
                psz_p = ss.enter_context(tc.tile_pool(
                    name=f"{name}_psz", bufs=2, space="PSUM"))
                psn_p = ss.enter_context(tc.tile_pool(
                    name=f"{name}_psn", bufs=2, space="PSUM"))

                for t in range(T):
                    h_cur, h_nxt = h_t[t % 2], h_t[(t + 1) % 2]
                    if gx_rows2 is None:
                        gx = gxp.tile([128, GW], f32, tag="gx")
                        nc.sync.dma_start(gx[:], gx_rows[:, t, :])
                    else:
                        ga = gxp.tile([128, GW], f32, tag="gxa")
                        nc.sync.dma_start(ga[:], gx_rows[:, t, :])
                        gb = gxp.tile([128, GW], f32, tag="gxb")
                        nc.sync.dma_start(gb[:], gx_rows2[:, T - 1 - t, :])
                        gx = gxp.tile([128, GW], f32, tag="gxs")
                        nc.vector.tensor_add(gx[:], ga[:], gb[:])
                    ps_r = psr_p.tile([128, 4 * BA], f32, tag="ps",
                                      name=f"{name}_psr_t")
                    ps_z = psz_p.tile([128, 4 * BA], f32, tag="ps",
                                      name=f"{name}_psz_t")
                    ps_n = psn_p.tile([128, 4 * BA], f32, tag="ps",
                                      name=f"{name}_psn_t")
                    for m in range(8):          # r chunks 0..3, z chunks 4..7
                        ps = ps_r if m < 4 else ps_z
                        dst = ps[:, BA * (m % 4):BA * (m % 4 + 1)]
                        for k in range(KH):
                            nc.tensor.matmul(
                                dst, whh_sb[:, k, 128 * m:128 * (m + 1)],
                                h_cur[:, BA * k:BA * (k + 1)],
                                start=(k == 0), stop=(k == KH - 1))
                    for j in range(4):          # n gate chunks
                        m = 8 + j
                        dst = ps_n[:, BA * j:BA * (j + 1)]
                        # redundant leading bias mms (start=True overwrites)
                        # act as PE warmers when there is no phase work.
                        for _ in range(warm if j == 0 else 0):
                            nc.tensor.matmul(
                                dst, bhn_sb[:, 0:128], ones_bf[:],
                                start=True, stop=False)
                        nc.tensor.matmul(
                            dst, bhn_sb[:, 128 * j:128 * (j + 1)],
                            ones_bf[:], start=True, stop=False)
                        for k in range(KH):
                            nc.tensor.matmul(
                                dst, whh_sb[:, k, 128 * m:128 * (m + 1)],
                                h_cur[:, BA * k:BA * (k + 1)],
                                start=False, stop=(k == KH - 1))
                    r_pre = gp.tile([128, 4 * BA], f32, tag="r_pre",
                                    name=f"{name}_rp")
                    nc.vector.tensor_add(r_pre[:], ps_r[:],
                                         gx[:, 0:4 * BA])
                    r = gp.tile([128, 4 * BA], f32, tag="r", name=f"{name}_r")
                    nc.scalar.activation(r[:], r_pre[:], SIG)
                    z_pre = gp.tile([128, 4 * BA], f32, tag="z_pre",
                                    name=f"{name}_zpre")
                    nc.vector.tensor_add(z_pre[:], ps_z[:],
                                         gx[:, 4 * BA:8 * BA])
                    z = gp.tile([128, 4 * BA], f32, tag="z", name=f"{name}_z")
                    nc.scalar.activation(z[:], z_pre[:], SIG)
                    zp = gp.tile([128, 4 * BA], f32, tag="zp",
                                 name=f"{name}_zp")
                    nc.scalar.activation(zp[:], z_pre[:], SIG, scale=-1.0)
                    n1 = gp.tile([128, 4 * BA], f32, tag="n1",
                                 name=f"{name}_n1")
                    nc.vector.tensor_mul(n1[:], ps_n[:], r[:])
                    n2 = gp.tile([128, 4 * BA], f32, tag="n2",
                                 name=f"{name}_n2")
                    nc.vector.tensor_add(n2[:], n1[:], gx[:, 8 * BA:12 * BA])
                    u = gp.tile([128, 4 * BA], f32, tag="u", name=f"{name}_u")
                    nc.vector.tensor_mul(u[:], z[:], h_cur[:])
                    nt = gp.tile([128, 4 * BA], f32, tag="nt",
                                 name=f"{name}_nt")
                    nc.scalar.activation(nt[:], n2[:], TANH)
                    v = gp.tile([128, 4 * BA], f32, tag="v", name=f"{name}_v")
                    nc.vector.tensor_mul(v[:], zp[:], nt[:])
                    # h' in halves: chunks {0,1} land first so the next
                    # step's k=0,1 matmuls can begin while {2,3} finish.
                    HB = 2 * BA
                    nc.vector.tensor_add(h_nxt[:, 0:HB], u[:, 0:HB],
                                         v[:, 0:HB])
                    nc.vector.tensor_add(h_nxt[:, HB:2 * HB], u[:, HB:2 * HB],
                                         v[:, HB:2 * HB])
                    if store_h is not None:
                        nc.gpsimd.dma_start(store_h[t, :, :], h_nxt[:])
                    if interleave is not None:
                        interleave(t)
            return h_t[T % 2]

        # ====== Phase A: layer-0 scan with interleaved phase work ======
        scan(h0a, h0b, gx0, None, whhT0_sb, bhn0_sb, hbuf, "s0",
             interleave=s0_interleave)
        for g in live_gens:   # safety: finish anything not fully pumped
            drain(g)

        # Exposed gap: the block-7 gx1 passes (b first: scan1 step 0
        # reads gx1b's last rows), then release the phase pools.
        drain(gx1_pass(1, NTB - 1))
        drain(gx1_pass(0, NTB - 1))
        ph_stack.close()

        # ================ layer-1 backward: single step at t = T-1 =========
        with contextlib.ExitStack() as lbs:
            lb_wp = lbs.enter_context(tc.tile_pool(name="lb_wp", bufs=1))
            l1b_w = lb_wp.tile([128, 2 * KH, 3 * H], bf16, tag="l1b_w")
            nc.sync.dma_start(l1b_w[:],
                              wih1bT.rearrange("(k p) m -> p k m", p=128))
            ag_last = ag_chunks[-1][2]      # rows 496..511
            ag_first = ag_chunks[0][2]      # rows 0..63
            for j in range(KH):
                nc.sync.dma_start(x1last[:, j, :],
                                  ag_last[0, 15, :, BA * j:BA * (j + 1)])
                nc.sync.dma_start(x1last[:, KH + j, :],
                                  ag_first[1, 0, :, BA * j:BA * (j + 1)])
            l1b_ps_p = lbs.enter_context(
                tc.tile_pool(name="l1b_ps", bufs=1, space="PSUM"))
            l1b_ps = l1b_ps_p.tile([128, MCH * BA], f32, tag="l1b_ps")
            for m in range(MCH):
                for k in range(2 * KH):
                    nc.tensor.matmul(l1b_ps[:, BA * m:BA * (m + 1)],
                                     l1b_w[:, k, 128 * m:128 * (m + 1)],
                                     x1last[:, k, :],
                                     start=(k == 0), stop=(k == 2 * KH - 1))
            for m in range(MCH):
                nc.vector.tensor_scalar_add(gxl[:, BA * m:BA * (m + 1)],
                                            l1b_ps[:, BA * m:BA * (m + 1)],
                                            b1b_sb[:, m:m + 1])
            nc.scalar.activation(rl[:], gxl[:, 0:4 * BA], SIG)
            nc.scalar.activation(zpl[:], gxl[:, 4 * BA:8 * BA], SIG, scale=-1.0)
            for jj in range(KH):
                nc.vector.scalar_tensor_tensor(
                    n1l[:, BA * jj:BA * (jj + 1)], rl[:, BA * jj:BA * (jj + 1)],
                    bhn1b_sb[:, jj:jj + 1],
                    gxl[:, 8 * BA + BA * jj:8 * BA + BA * (jj + 1)],
                    ALU.mult, ALU.add)
            nc.scalar.activation(ntl[:], n1l[:], TANH)
            nc.vector.tensor_mul(h1bk[:], zpl[:], ntl[:])

        # ================ Phase B: layer-1 forward scan ================
        h1f_bf = scan(h1a, h1b_, gx1f, gx1b, whh1_sb, bhn1_sb, None, "s1")
        h1f = const_tile([128, KH * BA], f32r, "h1f_r")
        nc.vector.tensor_copy(h1f[:], h1f_bf[:])

        # ================ FC ================
        with contextlib.ExitStack() as fcs:
            fc_ps_p = fcs.enter_context(
                tc.tile_pool(name="fc_ps", bufs=1, space="PSUM"))
            fc_ps_t = fc_ps_p.tile([128, BA], f32, tag="fc_ps", name="fc_ps")
            fc_ps = fc_ps_t[0:C, :]
            for k in range(KH):
                nc.tensor.matmul(fc_ps, fcw_sb[:, k, :],
                                 h1f[:, BA * k:BA * (k + 1)],
                                 start=(k == 0), stop=False)
            for k in range(KH):
                nc.tensor.matmul(fc_ps, fcw_sb[:, KH + k, :],
                                 h1bk[:, BA * k:BA * (k + 1)],
                                 start=False, stop=(k == KH - 1))
            nc.scalar.activation(out_sb, fc_ps, IDENT, bias=fcb_sb)
            nc.sync.dma_start(outT[:], out_sb)

    nc.compile()
    return nc


def _make_in_maps(inputs, T):
    x = np.asarray(inputs["x"], dtype=np.float32)

    import ml_dtypes
    bf = ml_dtypes.bfloat16

    def layer_params(wih, whh, bih, bhh):
        wih, whh = np.asarray(wih), np.asarray(whh)
        bih, bhh = np.asarray(bih), np.asarray(bhh)
        bias = (bih + bhh).astype(np.float32).copy()
        bias[2 * H:] = bih[2 * H:]
        return {
            "wihT": np.ascontiguousarray(wih.T, dtype=np.float32),
            "whhT": np.ascontiguousarray(whh.T).astype(bf),
            "bias": bias.reshape(1, 3 * H),
            "bhn": bhh[2 * H:].reshape(1, H).astype(bf),
        }

    l0f = layer_params(inputs["w_ih_l0f"], inputs["w_hh_l0f"],
                       inputs["b_ih_l0f"], inputs["b_hh_l0f"])
    l0b = layer_params(inputs["w_ih_l0b"], inputs["w_hh_l0b"],
                       inputs["b_ih_l0b"], inputs["b_hh_l0b"])
    l1f = layer_params(inputs["w_ih_l1f"], inputs["w_hh_l1f"],
                       inputs["b_ih_l1f"], inputs["b_hh_l1f"])

    wih1fT = np.ascontiguousarray(np.asarray(inputs["w_ih_l1f"]).T
                                  ).astype(bf)  # [2H, 3H]
    wih1bT = np.ascontiguousarray(np.asarray(inputs["w_ih_l1b"]).T).astype(bf)

    b1b = (np.asarray(inputs["b_ih_l1b"]) + np.asarray(inputs["b_hh_l1b"])
           ).astype(np.float32).copy()
    b1b[2 * H:] = np.asarray(inputs["b_ih_l1b"])[2 * H:]
    bias1b_sc = np.ascontiguousarray(b1b.reshape(MCH, 128).T)
    bhn1b_sc = np.ascontiguousarray(
        np.asarray(inputs["b_hh_l1b"])[2 * H:].reshape(KH, 128).T
        .astype(np.float32))

    fcwT = np.ascontiguousarray(np.asarray(inputs["fc_w"]).T, dtype=np.float32)
    fcb = np.asarray(inputs["fc_b"]).reshape(C, 1).astype(np.float32)

    common = {
        "wih1T_f": np.ascontiguousarray(wih1fT[:H]),
        "wih1T_b": np.ascontiguousarray(wih1fT[H:]),
        "bias1": l1f["bias"],
        "bhn1": l1f["bhn"],
        "whh1T": l1f["whhT"],
        "wih1bT": wih1bT,
        "bias1b_sc": bias1b_sc,
        "bhn1b_sc": bhn1b_sc,
        "fcwT": fcwT,
        "fcb": fcb,
    }

    in_maps = []
    for i in range(NCORES):
        p = i % 4
        back = i >= 4
        xs = x[8 * p:8 * p + 8, :T, :]
        if back:
            xs = xs[:, ::-1, :]
        xTl = np.ascontiguousarray(xs.transpose(2, 1, 0))  # [I, T, BA]
        lp = l0b if back else l0f
        m = {
            "xT": xTl,
            "wihT0": lp["wihT"],
            "bias0": lp["bias"],
            "bhn0": lp["bhn"],
            "whhT0": lp["whhT"],
        }
        m.update(common)
        in_maps.append(m)
    return in_maps


def _run(nc, in_maps, trace=False, trace_kwargs=None):
    from concourse.bass_utils import run_bass_kernel_spmd

    last_err = None
    for _ in range(3):
        try:
            return run_bass_kernel_spmd(nc, in_maps,
                                        core_ids=list(range(NCORES)),
                                        trace=trace,
                                        **(trace_kwargs or {}))
        except Exception as e:  # transient NRT device errors
            last_err = e
            import time
            time.sleep(5)
    raise last_err


def kernel(**inputs):
    T = np.asarray(inputs["x"]).shape[1]
    if T not in _PROGRAM_CACHE:
        _PROGRAM_CACHE[T] = _build(T)
    nc = _PROGRAM_CACHE[T]
    in_maps = _make_in_maps(inputs, T)
    res = _run(nc, in_maps)
    out = np.zeros((B, C), dtype=np.float32)
    for p in range(4):
        out[8 * p:8 * p + 8, :] = res.results[p]["outT"].T
    return out


# revision 25
# speedup vs baseline: 1.0448x; 1.0448x over previous
"""Bidirectional 2-layer GRU + FC kernel for Trainium2 (8 NeuronCores).

Strategy:
  - Cores 0-3 run layer-0 FORWARD for batch slice [8p, 8p+8); cores 4-7 run
    layer-0 BACKWARD for the same slices (fed time-reversed x + backward
    weights via per-core in_maps; the device program is SPMD-uniform).
  - gx (input projections + biases) are precomputed with big fp32r matmuls
    into a row-major DRAM layout [128, T, MCH*BA] so each scan step loads one
    contiguous 384B/partition row.
  - The recurrence runs in transposed layout: gate rows on partitions,
    batch on the free dim.  Per step: 48 bf16 matmuls (w_hh.T chunks) +
    4 bias matmuls, ordered r/z-gates first so the sigmoid overlaps the
    n-gate matmuls.  Gate math is full-width (6 DVE ops + 3 ACTs) using
    h' = z*h + (1-z)*n with (1-z) = sigmoid(-x).  rz and n PSUM
    accumulators live in different banks so DVE reads don't serialize
    against PE writes.
  - A pairwise AllGather {i, i+4} shares the layer-0 trajectories; both pair
    members then redundantly compute layer-1 forward for their 8 batches
    (recurrence wall-time is batch-independent, so redundancy is free).
  - Layer-1 backward contributes only its t=T-1 state to the output, which
    takes a single step from h0=0.  FC bias is fused into an ACT Identity.
"""

import contextlib

import numpy as np

B, T_FULL, I_IN, H, C = 32, 512, 256, 512, 10
NCORES = 8
BA = 8          # batch per core
MCH = 12        # 3H / 128 gate-row chunks
KH = 4          # H / 128 contraction chunks

_PROGRAM_CACHE = {}


def _build(T):
    import concourse.bacc as bacc
    import concourse.mybir as mybir
    import concourse.tile as tile

    f32 = mybir.dt.float32
    f32r = mybir.dt.float32r
    bf16 = mybir.dt.bfloat16
    SIG = mybir.ActivationFunctionType.Sigmoid
    TANH = mybir.ActivationFunctionType.Tanh
    IDENT = mybir.ActivationFunctionType.Identity
    ALU = mybir.AluOpType

    TB = min(64, T)          # timestep block for the big matmul phases
    NTB = T // TB
    GW = MCH * BA            # 96 gx row width

    nc = bacc.Bacc("TRN2", target_bir_lowering=False, debug=False,
                   num_devices=NCORES)

    def inp(name, shape, dt=f32r):
        return nc.dram_tensor(name, shape, dt, kind="ExternalInput").ap()

    xT = inp("xT", [I_IN, T, BA])               # own batch slice, own time order
    wihT0 = inp("wihT0", [I_IN, 3 * H])         # own direction w_ih.T
    bias0 = inp("bias0", [1, 3 * H])            # b_ih + b_hh (rz); n part = b_ih_n
    bhn0 = inp("bhn0", [1, H], bf16)                  # b_hh n part
    whhT0 = inp("whhT0", [H, 3 * H], bf16)
    wih1T_f = inp("wih1T_f", [H, 3 * H], bf16)        # w_ih_l1f.T rows 0:H   (f0 input)
    wih1T_b = inp("wih1T_b", [H, 3 * H], bf16)        # w_ih_l1f.T rows H:2H  (b0 input)
    bias1 = inp("bias1", [1, 3 * H])
    bhn1 = inp("bhn1", [1, H], bf16)
    whh1T = inp("whh1T", [H, 3 * H], bf16)
    wih1bT = inp("wih1bT", [2 * H, 3 * H], bf16)      # w_ih_l1b.T
    bias1b_sc = inp("bias1b_sc", [128, MCH], f32)   # per m-chunk column
    bhn1b_sc = inp("bhn1b_sc", [128, KH], f32)
    fcwT = inp("fcwT", [2 * H, C])
    fcb = inp("fcb", [C, 1], f32)

    outT = nc.dram_tensor("outT", [C, BA], f32, kind="ExternalOutput").ap()

    with tile.TileContext(nc) as tc, contextlib.ExitStack() as ctx:
        # ---------------- DRAM scratch (Tile-tracked) ----------------
        dramp = ctx.enter_context(tc.tile_pool(name="dramp", bufs=1, space="DRAM"))
        gx0 = dramp.tile([128, T, GW], f32, tag="gx0")
        hbuf = dramp.tile([T, 128, KH * BA], bf16, tag="hbuf")
        # AllGather lands in per-chunk tiles (contiguous in/out required).
        # 7 x 64 rows + 48 + 16: the small final chunks shrink the exposed
        # gap between scan0's last step and layer-1 work that needs t=T-1.
        _ag_bounds = [(64 * i, 64) for i in range(7)] + [(448, 48), (496, 16)]
        ag_chunks = [
            (s, L, dramp.tile([2, L, 128, KH * BA], bf16, tag=f"agc{i}",
                              name=f"agc{i}"))
            for i, (s, L) in enumerate(_ag_bounds)]
        gx1f = dramp.tile([128, T, GW], f32, tag="gx1f")
        gx1b = dramp.tile([128, T, GW], f32, tag="gx1b")

        # ---------------- persistent SBUF (one pool, distinct tags) --------
        constp = ctx.enter_context(tc.tile_pool(name="constp", bufs=1))

        def const_tile(shape, dt, tag):
            return constp.tile(shape, dt, tag=tag, name=tag)

        whhT0_sb = const_tile([128, KH, 3 * H], bf16, "whhT0_sb")
        nc.sync.dma_start(whhT0_sb[:], whhT0.rearrange("(k p) m -> p k m", p=128))
        bias0_sb = const_tile([128, 3 * H], f32r, "bias0_sb")[0:1, :]
        nc.sync.dma_start(bias0_sb, bias0[:])
        # b_hh_n as a zero-padded [128, H] weight tile (data in partition 0
        # only): the bias matmul's LDWEIGHTS is then a full 128-col load
        # that pipelines like the others (a [1,128] load can't FWL).
        bhn0_sb = const_tile([128, H], bf16, "bhn0_sb")
        nc.vector.memset(bhn0_sb[:], 0.0)
        nc.sync.dma_start(bhn0_sb[0:1, :], bhn0[:])
        ones_big = const_tile([128, TB * BA], f32, "ones_big")[0:1, :]
        nc.vector.memset(ones_big, 1.0)
        ones_bf = const_tile([128, BA], bf16, "ones_bf")
        nc.vector.memset(ones_bf[:], 1.0)

        whh1_sb = const_tile([128, KH, 3 * H], bf16, "whh1_sb")
        nc.sync.dma_start(whh1_sb[:], whh1T.rearrange("(k p) m -> p k m", p=128))
        bhn1_sb = const_tile([128, H], bf16, "bhn1_sb")
        nc.vector.memset(bhn1_sb[:], 0.0)
        nc.sync.dma_start(bhn1_sb[0:1, :], bhn1[:])
        bias1_sb = const_tile([128, 3 * H], f32r, "bias1_sb")[0:1, :]
        nc.sync.dma_start(bias1_sb, bias1[:])

        b1b_sb = const_tile([128, MCH], f32, "b1b_sb")
        nc.sync.dma_start(b1b_sb[:], bias1b_sc[:])
        bhn1b_sb = const_tile([128, KH], f32, "bhn1b_sb")
        nc.sync.dma_start(bhn1b_sb[:], bhn1b_sc[:])
        fcw_sb = const_tile([128, 2 * KH, C], f32r, "fcw_sb")
        nc.sync.dma_start(fcw_sb[:], fcwT.rearrange("(k p) c -> p k c", p=128))
        fcb_sb = const_tile([128, 1], f32, "fcb_sb")[0:C, :]
        nc.sync.dma_start(fcb_sb, fcb[:])

        # h-state tiles for both scans + l1b results (long-lived)
        h0a = const_tile([128, KH * BA], bf16, "h0a")
        h0b = const_tile([128, KH * BA], bf16, "h0b")
        h1a = const_tile([128, KH * BA], bf16, "h1a")
        h1b_ = const_tile([128, KH * BA], bf16, "h1b_")
        gxl = const_tile([128, MCH * BA], f32, "gxl")
        rl = const_tile([128, 4 * BA], f32, "rl")
        zpl = const_tile([128, 4 * BA], f32, "zpl")
        n1l = const_tile([128, 4 * BA], f32, "n1l")
        ntl = const_tile([128, 4 * BA], f32, "ntl")
        h1bk = const_tile([128, 4 * BA], f32r, "h1bk")
        x1last = const_tile([128, 2 * KH, BA], bf16, "x1last")
        out_sb = const_tile([128, BA], f32, "out_sb")[0:C, :]

        # Scan-hot pools allocated LOW in SBUF (before the big phase
        # tiles) so the per-step DVE/ACT operands stay clear of the PE
        # weight-stream regions.
        scan_pools = {}
        for nm in ("s0", "s1"):
            scan_pools[nm] = (
                ctx.enter_context(tc.tile_pool(name=f"{nm}_gx", bufs=6)),
                ctx.enter_context(tc.tile_pool(name=f"{nm}_g", bufs=3)),
            )

        # ======== Phase machinery: gx0 / AllGather / gx1 as generators ======
        # All big-matmul phase work is chopped into quanta (one mm / copy /
        # DMA per yield) and pumped from inside the scan0 loop, filling the
        # PE idle window of each recurrence step and hiding the AllGather.
        wih0_sb = const_tile([128, 2, 3 * H], f32r, "wih0_sb")
        nc.sync.dma_start(wih0_sb[:],
                          wihT0.rearrange("(k p) m -> p k m", p=128))
        w1f_sb = const_tile([128, KH, 3 * H], bf16, "w1f_sb")
        nc.sync.dma_start(w1f_sb[:],
                          wih1T_f.rearrange("(k p) m -> p k m", p=128))
        w1b_sb = const_tile([128, KH, 3 * H], bf16, "w1b_sb")
        nc.sync.dma_start(w1b_sb[:],
                          wih1T_b.rearrange("(k p) m -> p k m", p=128))

        ph_stack = contextlib.ExitStack()
        ph_mv0 = ph_stack.enter_context(tc.tile_pool(name="ph_mv0", bufs=2))
        ph_mv1 = ph_stack.enter_context(tc.tile_pool(name="ph_mv1", bufs=3))
        ph_ps = ph_stack.enter_context(tc.tile_pool(name="ph_ps", bufs=2,
                                                    space="PSUM"))
        ph_ot = ph_stack.enter_context(tc.tile_pool(name="ph_ot", bufs=3))

        xT_r = xT.rearrange("(k p) t b -> p k t b", p=128)

        chain_cell = {"last": None}

        def _copy_m(ot, ps, m):
            # deferred by one m-group when emitted (matmul sems posted),
            # and ordered after the current step's final h' write so the
            # scheduler can't hoist it into the scan's critical chain.
            dst = ot[:, :, BA * m:BA * (m + 1)]
            src = ps[:].rearrange("p (t b) -> p t b", t=TB)
            ret = nc.vector.tensor_copy(dst, src)
            if chain_cell["last"] is not None:
                tile.add_dep_helper(ret.ins, chain_cell["last"].ins,
                                    sync=False, reason="phase copy after h'")

        def phase0_block(c):
            mv = ph_mv0.tile([128, 2, TB, BA], f32r, tag="mv")
            nc.sync.dma_start(mv[:], xT_r[:, :, c * TB:(c + 1) * TB, :])
            yield
            ot = ph_ot.tile([128, TB, GW], f32, tag="ot")
            prev = None
            for m in range(MCH):
                ps = ph_ps.tile([128, TB * BA], f32, tag="ps")
                for k in range(2):
                    nc.tensor.matmul(ps[:],
                                     wih0_sb[:, k, 128 * m:128 * (m + 1)],
                                     mv[:, k, :, :],
                                     start=(k == 0), stop=False)
                    yield
                nc.tensor.matmul(ps[:], bias0_sb[:, 128 * m:128 * (m + 1)],
                                 ones_big.bitcast(f32r),
                                 start=False, stop=True)
                yield
                if prev is not None:
                    _copy_m(ot, prev[0], prev[1])
                    yield
                prev = (ps, m)
            _copy_m(ot, prev[0], prev[1])
            yield
            nc.scalar.dma_start(gx0[:, c * TB:(c + 1) * TB, :], ot[:])
            yield

        def gx1_pass(pi, c):
            w_sb = w1f_sb if pi == 0 else w1b_sb
            dstbuf = gx1f if pi == 0 else gx1b
            with_bias = pi == 0
            mv = ph_mv1.tile([128, TB, KH * BA], bf16, tag="g1mv")
            # gather the 64-row block from the (possibly split) AG chunks
            r0 = c * TB
            for (s, L, agt) in ag_chunks:
                lo, hi = max(r0, s), min(r0 + TB, s + L)
                if lo < hi:
                    nc.sync.dma_start(
                        mv[:, lo - r0:hi - r0, :],
                        agt[pi, lo - s:hi - s].rearrange("t p b -> p t b"))
            yield
            ot = ph_ot.tile([128, TB, GW], f32, tag="ot")
            prev = None
            for m in range(MCH):
                ps = ph_ps.tile([128, TB * BA], f32, tag="ps")
                for k in range(KH):
                    nc.tensor.matmul(
                        ps[:], w_sb[:, k, 128 * m:128 * (m + 1)],
                        mv[:, :, BA * k:BA * (k + 1)],
                        start=(k == 0),
                        stop=(not with_bias and k == KH - 1))
                    yield
                if with_bias:
                    nc.tensor.matmul(
                        ps[:], bias1_sb[:, 128 * m:128 * (m + 1)],
                        ones_big.bitcast(f32r), start=False, stop=True)
                    yield
                if prev is not None:
                    _copy_m(ot, prev[0], prev[1])
                    yield
                prev = (ps, m)
            _copy_m(ot, prev[0], prev[1])
            yield
            nc.scalar.dma_start(
                dstbuf[:, c * TB:(c + 1) * TB, :], ot[:])
            yield

        def allgather_chunk(idx):
            s, L, agt = ag_chunks[idx]
            nc.gpsimd.collective_compute(
                "AllGather", ALU.bypass,
                replica_groups=[[0, 4], [1, 5], [2, 6], [3, 7]],
                ins=[hbuf[s:s + L]],
                outs=[agt[:]])

        def drain(gen):
            for _ in gen:
                pass

        # Static schedule: step index -> work pumped after that step's body.
        pump = {}

        def register(gen, t0, qps, total):
            steps = (total + qps - 1) // qps
            for i in range(steps):
                pump.setdefault(t0 + i, []).append((gen, qps))

        # phase0 blocks 0,1 run up-front (block 0 gates scan0 step 0).
        drain(phase0_block(0))
        drain(phase0_block(1))
        live_gens = []

        def track(gen):
            live_gens.append(gen)
            return gen

        for c in range(2, NTB):
            register(track(phase0_block(c)), 64 * (c - 2) + 24, 2, 52)
        ag_at = {s + L - 1: i for i, (s, L, _) in enumerate(ag_chunks)}
        for c in range(NTB - 1):
            t0 = 64 * c + 84 if c < 6 else 460
            register(track(gx1_pass(0, c)), t0, 2, 80)
            register(track(gx1_pass(1, c)), t0 + 20, 2, 70)

        def s0_interleave(t):
            for gen, n in pump.get(t, ()):
                for _ in range(n):
                    if next(gen, StopIteration) is StopIteration:
                        break
            if t in ag_at:
                allgather_chunk(ag_at[t])

        # ================ generic GRU scan (full-width pipeline) ============
        # PE order: r/z gate chunks (m 0..7) first, then n chunks (m 8..11)
        # + b_hh_n bias.  Full-width gate math:
        #   rz = sig(ps_rz + gx_rz); zp = sig(-(z_pre));
        #   n = tanh(ps_n * r + gx_n); h' = z*h + zp*n.
        def scan(h_even, h_odd, gx_rows, gx_rows2, whh_sb, bhn_sb,
                 store_h, name, interleave=None, warm=0):
            h_t = [h_even, h_odd]
            nc.vector.memset(h_t[0][:], 0.0)
            gxp, gp = scan_pools[name]
            with contextlib.ExitStack() as ss:
                psr_p = ss.enter_context(tc.tile_pool(
                    name=f"{name}_psr", bufs=2, space="PSUM"))
                psz_p = ss.enter_context(tc.tile_pool(
                    name=f"{name}_psz", bufs=2, space="PSUM"))
                psn_p = ss.enter_context(tc.tile_pool(
                    name=f"{name}_psn", bufs=2, space="PSUM"))

                for t in range(T):
                    h_cur, h_nxt = h_t[t % 2], h_t[(t + 1) % 2]
                    if gx_rows2 is None:
                        gx = gxp.tile([128, GW], f32, tag="gx")
                        nc.sync.dma_start(gx[:], gx_rows[:, t, :])
                    else:
                        ga = gxp.tile([128, GW], f32, tag="gxa")
                        nc.sync.dma_start(ga[:], gx_rows[:, t, :])
                        gb = gxp.tile([128, GW], f32, tag="gxb")
                        nc.sync.dma_start(gb[:], gx_rows2[:, T - 1 - t, :])
                        gx = gxp.tile([128, GW], f32, tag="gxs")
                        nc.vector.tensor_add(gx[:], ga[:], gb[:])
                    ps_r = psr_p.tile([128, 4 * BA], f32, tag="ps",
                                      name=f"{name}_psr_t")
                    ps_z = psz_p.tile([128, 4 * BA], f32, tag="ps",
                                      name=f"{name}_psz_t")
                    ps_n = psn_p.tile([128, 4 * BA], f32, tag="ps",
                                      name=f"{name}_psn_t")
                    for m in range(8):          # r chunks 0..3, z chunks 4..7
                        ps = ps_r if m < 4 else ps_z
                        dst = ps[:, BA * (m % 4):BA * (m % 4 + 1)]
                        for k in range(KH):
                            nc.tensor.matmul(
                                dst, whh_sb[:, k, 128 * m:128 * (m + 1)],
                                h_cur[:, BA * k:BA * (k + 1)],
                                start=(k == 0), stop=(k == KH - 1))
                    for j in range(4):          # n gate chunks
                        m = 8 + j
                        dst = ps_n[:, BA * j:BA * (j + 1)]
                        # redundant leading bias mms (start=True overwrites)
                        # act as PE warmers when there is no phase work.
                        for _ in range(warm if j == 0 else 0):
                            nc.tensor.matmul(
                                dst, bhn_sb[:, 0:128], ones_bf[:],
                                start=True, stop=False)
                        nc.tensor.matmul(
                            dst, bhn_sb[:, 128 * j:128 * (j + 1)],
                            ones_bf[:], start=True, stop=False)
                        for k in range(KH):
                            nc.tensor.matmul(
                                dst, whh_sb[:, k, 128 * m:128 * (m + 1)],
                                h_cur[:, BA * k:BA * (k + 1)],
                                start=False, stop=(k == KH - 1))
                    r_pre = gp.tile([128, 4 * BA], f32, tag="r_pre",
                                    name=f"{name}_rp")
                    nc.vector.tensor_add(r_pre[:], ps_r[:],
                                         gx[:, 0:4 * BA])
                    r = gp.tile([128, 4 * BA], f32, tag="r", name=f"{name}_r")
                    nc.scalar.activation(r[:], r_pre[:], SIG)
                    z_pre = gp.tile([128, 4 * BA], f32, tag="z_pre",
                                    name=f"{name}_zpre")
                    nc.vector.tensor_add(z_pre[:], ps_z[:],
                                         gx[:, 4 * BA:8 * BA])
                    z = gp.tile([128, 4 * BA], f32, tag="z", name=f"{name}_z")
                    nc.scalar.activation(z[:], z_pre[:], SIG)
                    zp = gp.tile([128, 4 * BA], f32, tag="zp",
                                 name=f"{name}_zp")
                    nc.scalar.activation(zp[:], z_pre[:], SIG, scale=-1.0)
                    n1 = gp.tile([128, 4 * BA], f32, tag="n1",
                                 name=f"{name}_n1")
                    nc.vector.tensor_mul(n1[:], ps_n[:], r[:])
                    n2 = gp.tile([128, 4 * BA], f32, tag="n2",
                                 name=f"{name}_n2")
                    nc.vector.tensor_add(n2[:], n1[:], gx[:, 8 * BA:12 * BA])
                    u = gp.tile([128, 4 * BA], f32, tag="u", name=f"{name}_u")
                    nc.vector.tensor_mul(u[:], z[:], h_cur[:])
                    nt = gp.tile([128, 4 * BA], f32, tag="nt",
                                 name=f"{name}_nt")
                    nc.scalar.activation(nt[:], n2[:], TANH)
                    v = gp.tile([128, 4 * BA], f32, tag="v", name=f"{name}_v")
                    nc.vector.tensor_mul(v[:], zp[:], nt[:])
                    # h' in halves: chunks {0,1} land first so the next
                    # step's k=0,1 matmuls can begin while {2,3} finish.
                    HB = 2 * BA
                    nc.vector.tensor_add(h_nxt[:, 0:HB], u[:, 0:HB],
                                         v[:, 0:HB])
                    hb_ret = nc.vector.tensor_add(h_nxt[:, HB:2 * HB],
                                                  u[:, HB:2 * HB],
                                                  v[:, HB:2 * HB])
                    if chain_cell is not None:
                        chain_cell["last"] = hb_ret
                    if store_h is not None:
                        nc.gpsimd.dma_start(store_h[t, :, :], h_nxt[:])
                    if interleave is not None:
                        interleave(t)
            return h_t[T % 2]

        # ====== Phase A: layer-0 scan with interleaved phase work ======
        scan(h0a, h0b, gx0, None, whhT0_sb, bhn0_sb, hbuf, "s0",
             interleave=s0_interleave)
        for g in live_gens:   # safety: finish anything not fully pumped
            drain(g)

        # Exposed gap: the block-7 gx1 passes (b first: scan1 step 0
        # reads gx1b's last rows), then release the phase pools.
        drain(gx1_pass(1, NTB - 1))
        drain(gx1_pass(0, NTB - 1))
        ph_stack.close()

        # ================ layer-1 backward: single step at t = T-1 =========
        with contextlib.ExitStack() as lbs:
            lb_wp = lbs.enter_context(tc.tile_pool(name="lb_wp", bufs=1))
            l1b_w = lb_wp.tile([128, 2 * KH, 3 * H], bf16, tag="l1b_w")
            nc.sync.dma_start(l1b_w[:],
                              wih1bT.rearrange("(k p) m -> p k m", p=128))
            ag_last = ag_chunks[-1][2]      # rows 496..511
            ag_first = ag_chunks[0][2]      # rows 0..63
            for j in range(KH):
                nc.sync.dma_start(x1last[:, j, :],
                                  ag_last[0, 15, :, BA * j:BA * (j + 1)])
                nc.sync.dma_start(x1last[:, KH + j, :],
                                  ag_first[1, 0, :, BA * j:BA * (j + 1)])
            l1b_ps_p = lbs.enter_context(
                tc.tile_pool(name="l1b_ps", bufs=1, space="PSUM"))
            l1b_ps = l1b_ps_p.tile([128, MCH * BA], f32, tag="l1b_ps")
            for m in range(MCH):
                for k in range(2 * KH):
                    nc.tensor.matmul(l1b_ps[:, BA * m:BA * (m + 1)],
                                     l1b_w[:, k, 128 * m:128 * (m + 1)],
                                     x1last[:, k, :],
                                     start=(k == 0), stop=(k == 2 * KH - 1))
            for m in range(MCH):
                nc.vector.tensor_scalar_add(gxl[:, BA * m:BA * (m + 1)],
                                            l1b_ps[:, BA * m:BA * (m + 1)],
                                            b1b_sb[:, m:m + 1])
            nc.scalar.activation(rl[:], gxl[:, 0:4 * BA], SIG)
            nc.scalar.activation(zpl[:], gxl[:, 4 * BA:8 * BA], SIG, scale=-1.0)
            for jj in range(KH):
                nc.vector.scalar_tensor_tensor(
                    n1l[:, BA * jj:BA * (jj + 1)], rl[:, BA * jj:BA * (jj + 1)],
                    bhn1b_sb[:, jj:jj + 1],
                    gxl[:, 8 * BA + BA * jj:8 * BA + BA * (jj + 1)],
                    ALU.mult, ALU.add)
            nc.scalar.activation(ntl[:], n1l[:], TANH)
            nc.vector.tensor_mul(h1bk[:], zpl[:], ntl[:])

        # ================ Phase B: layer-1 forward scan ================
        h1f_bf = scan(h1a, h1b_, gx1f, gx1b, whh1_sb, bhn1_sb, None, "s1")
        h1f = const_tile([128, KH * BA], f32r, "h1f_r")
        nc.vector.tensor_copy(h1f[:], h1f_bf[:])

        # ================ FC ================
        with contextlib.ExitStack() as fcs:
            fc_ps_p = fcs.enter_context(
                tc.tile_pool(name="fc_ps", bufs=1, space="PSUM"))
            fc_ps_t = fc_ps_p.tile([128, BA], f32, tag="fc_ps", name="fc_ps")
            fc_ps = fc_ps_t[0:C, :]
            for k in range(KH):
                nc.tensor.matmul(fc_ps, fcw_sb[:, k, :],
                                 h1f[:, BA * k:BA * (k + 1)],
                                 start=(k == 0), stop=False)
            for k in range(KH):
                nc.tensor.matmul(fc_ps, fcw_sb[:, KH + k, :],
                                 h1bk[:, BA * k:BA * (k + 1)],
                                 start=False, stop=(k == KH - 1))
            nc.scalar.activation(out_sb, fc_ps, IDENT, bias=fcb_sb)
            nc.sync.dma_start(outT[:], out_sb)

    nc.compile()
    return nc


def _make_in_maps(inputs, T):
    x = np.asarray(inputs["x"], dtype=np.float32)

    import ml_dtypes
    bf = ml_dtypes.bfloat16

    def layer_params(wih, whh, bih, bhh):
        wih, whh = np.asarray(wih), np.asarray(whh)
        bih, bhh = np.asarray(bih), np.asarray(bhh)
        bias = (bih + bhh).astype(np.float32).copy()
        bias[2 * H:] = bih[2 * H:]
        return {
            "wihT": np.ascontiguousarray(wih.T, dtype=np.float32),
            "whhT": np.ascontiguousarray(whh.T).astype(bf),
            "bias": bias.reshape(1, 3 * H),
            "bhn": bhh[2 * H:].reshape(1, H).astype(bf),
        }

    l0f = layer_params(inputs["w_ih_l0f"], inputs["w_hh_l0f"],
                       inputs["b_ih_l0f"], inputs["b_hh_l0f"])
    l0b = layer_params(inputs["w_ih_l0b"], inputs["w_hh_l0b"],
                       inputs["b_ih_l0b"], inputs["b_hh_l0b"])
    l1f = layer_params(inputs["w_ih_l1f"], inputs["w_hh_l1f"],
                       inputs["b_ih_l1f"], inputs["b_hh_l1f"])

    wih1fT = np.ascontiguousarray(np.asarray(inputs["w_ih_l1f"]).T
                                  ).astype(bf)  # [2H, 3H]
    wih1bT = np.ascontiguousarray(np.asarray(inputs["w_ih_l1b"]).T).astype(bf)

    b1b = (np.asarray(inputs["b_ih_l1b"]) + np.asarray(inputs["b_hh_l1b"])
           ).astype(np.float32).copy()
    b1b[2 * H:] = np.asarray(inputs["b_ih_l1b"])[2 * H:]
    bias1b_sc = np.ascontiguousarray(b1b.reshape(MCH, 128).T)
    bhn1b_sc = np.ascontiguousarray(
        np.asarray(inputs["b_hh_l1b"])[2 * H:].reshape(KH, 128).T
        .astype(np.float32))

    fcwT = np.ascontiguousarray(np.asarray(inputs["fc_w"]).T, dtype=np.float32)
    fcb = np.asarray(inputs["fc_b"]).reshape(C, 1).astype(np.float32)

    common = {
        "wih1T_f": np.ascontiguousarray(wih1fT[:H]),
        "wih1T_b": np.ascontiguousarray(wih1fT[H:]),
        "bias1": l1f["bias"],
        "bhn1": l1f["bhn"],
        "whh1T": l1f["whhT"],
        "wih1bT": wih1bT,
        "bias1b_sc": bias1b_sc,
        "bhn1b_sc": bhn1b_sc,
        "fcwT": fcwT,
        "fcb": fcb,
    }

    in_maps = []
    for i in range(NCORES):
        p = i % 4
        back = i >= 4
        xs = x[8 * p:8 * p + 8, :T, :]
        if back:
            xs = xs[:, ::-1, :]
        xTl = np.ascontiguousarray(xs.transpose(2, 1, 0))  # [I, T, BA]
        lp = l0b if back else l0f
        m = {
            "xT": xTl,
            "wihT0": lp["wihT"],
            "bias0": lp["bias"],
            "bhn0": lp["bhn"],
            "whhT0": lp["whhT"],
        }
        m.update(common)
        in_maps.append(m)
    return in_maps


def _run(nc, in_maps, trace=False, trace_kwargs=None):
    from concourse.bass_utils import run_bass_kernel_spmd

    last_err = None
    for _ in range(3):
        try:
            return run_bass_kernel_spmd(nc, in_maps,
                                        core_ids=list(range(NCORES)),
                                        trace=trace,
                                        **(trace_kwargs or {}))
        except Exception as e:  # transient NRT device errors
            last_err = e
            import time
            time.sleep(5)
    raise last_err


def kernel(**inputs):
    T = np.asarray(inputs["x"]).shape[1]
    if T not in _PROGRAM_CACHE:
        _PROGRAM_CACHE[T] = _build(T)
    nc = _PROGRAM_CACHE[T]
    in_maps = _make_in_maps(inputs, T)
    res = _run(nc, in_maps)
    out = np.zeros((B, C), dtype=np.float32)
    for p in range(4):
        out[8 * p:8 * p + 8, :] = res.results[p]["outT"].T
    return out
